# revision 14
# baseline (speedup 1.0000x reference)
"""Bahdanau-attention kernel for Trainium2 (8 NeuronCores, SPMD data-parallel).

Math (per batch b):
    dec_proj = decoder_hidden @ W[:1024] + b            # [A]
    z_T[a,s] = sum_e W[1024+e, a] * enc[b, s, e]        # [A, S] on PE
    energy   = tanh(z_T + dec_proj[:, None])            # ACT, per-partition bias
    logits   = sum_a v[a] * energy[a, s]                # PE matvec (v replicated)
    out[b]   = softmax(logits)                          # fp32 on ACT/DVE

Sharding: batch 32 -> 4 per core, weights replicated.
PE_DT picks the PE input dtype: "f16" (fp16, RNE ~2^-11) or "f32r" (FP22).
"""

import contextlib

import numpy as np

B, S, E, A, H = 32, 1024, 1024, 1024, 1024
NCORES = 8
BLOC = B // NCORES  # 4
KC = 8  # 128-chunks along each 1024-sized dim

PE_DT = "f16"

_PROGRAM_CACHE = {}


def _build_program(loop_iters=None, pe_dt=None):
    import concourse.bass as bass
    import concourse.tile as tile
    from concourse import bacc, mybir

    if pe_dt is None:
        pe_dt = PE_DT
    f32 = mybir.dt.float32
    f32r = mybir.dt.float32r
    f16 = mybir.dt.float16
    TANH = mybir.ActivationFunctionType.Tanh
    EXP = mybir.ActivationFunctionType.Exp
    AX = mybir.AxisListType.X
    OPMAX = mybir.AluOpType.max
    ts = bass.ts

    is16 = pe_dt == "f16"
    pdt = f16 if is16 else f32r

    nc = bacc.Bacc("TRN2", target_bir_lowering=False, debug=False)

    enc = nc.dram_tensor("enc", [BLOC, S, E], f32, kind="ExternalInput").ap()
    dec = nc.dram_tensor("dec", [BLOC, H], f32, kind="ExternalInput").ap()
    Wt = nc.dram_tensor("W", [H + E, A], f32, kind="ExternalInput").ap()
    bias = nc.dram_tensor("bias", [A], f32, kind="ExternalInput").ap()
    vvec = nc.dram_tensor("v", [A], f32, kind="ExternalInput").ap()
    out = nc.dram_tensor("out", [BLOC, S], f32, kind="ExternalOutput").ap()

    def w_src(ap2d):
        # fp16 path casts during SWDGE DMA; f32r path bitcasts bytes.
        return ap2d if is16 else ap2d.bitcast(f32r)

    def w_dma(o, i):
        if is16:
            nc.gpsimd.dma_start(out=o, in_=i)
        else:
            nc.sync.dma_start(out=o, in_=i)

    with tile.TileContext(nc) as tc:
        loop_cm = (
            tc.For_i(0, loop_iters, 1)
            if loop_iters is not None
            else contextlib.nullcontext()
        )
        with (
            tc.tile_pool(name="consts", bufs=1) as consts,
            tc.tile_pool(name="wenc", bufs=1) as wencp,
            tc.tile_pool(name="bigbuf", bufs=2) as bigbuf,
            tc.tile_pool(name="encT", bufs=2) as encTp,
            tc.tile_pool(name="tanh", bufs=3) as tanhp,
            tc.tile_pool(name="small", bufs=8) as small,
            tc.tile_pool(name="psz", bufs=3, space="PSUM") as pszp,
            tc.tile_pool(name="psatt", bufs=1, space="PSUM") as psattp,
        ):
            with loop_cm:
                # decT[p, h8, b] = dec[b, h8*128 + p]
                decT = consts.tile([128, KC, BLOC], pdt)
                dec_r = (dec if is16 else dec.bitcast(f32r)).rearrange(
                    "b (h8 p) -> p h8 b", p=128
                )
                for h8 in range(KC):
                    nc.gpsimd.dma_start(out=decT[:, h8, :], in_=dec_r[:, h8, :])
                # bias column per a-chunk: bcol[p, a8] = bias[a8*128 + p]
                bcol = consts.tile([128, KC], f32)
                nc.gpsimd.dma_start(
                    out=bcol, in_=bias.rearrange("(a8 p) -> p a8", p=128)
                )
                # v as columns, replicated across 128 cols per a-chunk
                vcol = consts.tile([128, KC], f32)
                nc.gpsimd.dma_start(
                    out=vcol, in_=vvec.rearrange("(a8 p) -> p a8", p=128)
                )
                ones = consts.tile([128, 128], f32)
                nc.vector.memset(ones, 1.0)
                # staging copy absorbs the DMA wait so tensor_scalar ops
                # below carry at most one sync-wait each
                vcol2 = consts.tile([128, KC], f32)
                nc.vector.tensor_copy(vcol2, vcol)
                vrep = consts.tile([128, KC, 128], pdt)
                for a8 in range(KC):
                    nc.vector.tensor_scalar_mul(
                        vrep[:, a8, :], ones, vcol2[:, a8 : a8 + 1]
                    )

                dproj = consts.tile([128, KC, BLOC], f32)

                # ---- batch-0 encoder load ------------------------------
                enc_src = enc if is16 else enc.bitcast(f32r)

                def enc_dma(o, i):
                    if is16:
                        nc.gpsimd.dma_start(out=o, in_=i)
                    else:
                        nc.sync.dma_start(out=o, in_=i)

                enc_nat = [None] * BLOC
                enc_nat[0] = bigbuf.tile(
                    [128, KC, E], pdt, tag="big", name="enc_nat0"
                )
                enc_r0 = enc_src[0].rearrange("(j p) e -> p j e", p=128)
                for s8 in range(KC):
                    enc_dma(enc_nat[0][:, s8, :], enc_r0[:, s8, :])

                # ---- W_enc ---------------------------------------------
                wenc = wencp.tile([128, KC, A], pdt)
                for e8 in range(KC):
                    w_dma(
                        wenc[:, e8, :],
                        w_src(Wt[H + e8 * 128 : H + (e8 + 1) * 128, :]),
                    )

                # ---- W_dec + dec_proj ----------------------------------
                if is16:
                    wdec = consts.tile([128, KC, A], pdt, name="wdec")
                else:
                    wdec = bigbuf.tile([128, KC, A], pdt, tag="big", name="wdec")
                for h8 in range(KC):
                    w_dma(wdec[:, h8, :], w_src(Wt[ts(h8, 128), :]))
                dp_ps = pszp.tile([128, KC, BLOC], f32, tag="z", name="dp_ps")
                for a8 in range(KC):
                    for h8 in range(KC):
                        nc.tensor.matmul(
                            dp_ps[:, a8, :],
                            wdec[:, h8, ts(a8, 128)],
                            decT[:, h8, :],
                            start=(h8 == 0),
                            stop=(h8 == KC - 1),
                        )
                for a8 in range(KC):
                    nc.vector.tensor_scalar_add(
                        dproj[:, a8, :], dp_ps[:, a8, :], bcol[:, a8 : a8 + 1]
                    )

                # ---- main batch loop -----------------------------------
                def emit_transpose_block(b, encT, s8):
                    # xbar DMA transpose: encT[p, j, s8*128+s'] =
                    #   enc_nat[b][s', s8, j*128+p]  (row e = j*128+p)
                    nc.sync.dma_start(
                        out=encT[:, :, ts(s8, 128)],
                        in_=enc_nat[b][:, s8, :],
                        transpose=True,
                    )

                def alloc_encT(b):
                    return encTp.tile(
                        [128, KC, S], pdt, tag="encT", name=f"eT{b}"
                    )

                # batch 0 transposes up front (chunk-JIT with its DMA)
                encT_cur = alloc_encT(0)
                for s8 in range(KC):
                    emit_transpose_block(0, encT_cur, s8)

                for b in range(BLOC):
                    if b + 1 < BLOC:
                        enc_nat[b + 1] = bigbuf.tile(
                            [128, KC, E], pdt, tag="big", name=f"enc_nat{b + 1}"
                        )
                        enc_rn = enc_src[b + 1].rearrange(
                            "(j p) e -> p j e", p=128
                        )
                        for s8 in range(KC):
                            enc_dma(enc_nat[b + 1][:, s8, :], enc_rn[:, s8, :])
                        encT_next = alloc_encT(b + 1)
                    else:
                        encT_next = None

                    att_ps = psattp.tile([128, S], f32, tag="att")

                    for a8 in range(KC):
                        # interleave next batch's transposes into this
                        # batch's z-phase (keeps PE dense / HAM warm)
                        if encT_next is not None:
                            emit_transpose_block(b + 1, encT_next, a8)
                        zp = pszp.tile([128, S], f32, tag="z")
                        for e8 in range(KC):
                            lhsT = wenc[:, e8, ts(a8, 128)]
                            nc.tensor.matmul(
                                zp[:, 0:512],
                                lhsT,
                                encT_cur[:, e8, 0:512],
                                start=(e8 == 0),
                                stop=(e8 == KC - 1),
                            )
                            nc.tensor.matmul(
                                zp[:, 512:1024],
                                lhsT,
                                encT_cur[:, e8, 512:1024],
                                start=(e8 == 0),
                                stop=(e8 == KC - 1),
                            )
                        tanh_t = tanhp.tile([128, S], pdt, tag="tanh")
                        nc.scalar.activation(
                            tanh_t,
                            zp,
                            TANH,
                            bias=dproj[:, a8, b : b + 1],
                            scale=1.0,
                        )
                        for half in (0, 512):
                            nc.tensor.matmul(
                                att_ps[:, half : half + 512],
                                vrep[:, a8, :],
                                tanh_t[:, half : half + 512],
                                start=(a8 == 0),
                                stop=(a8 == KC - 1),
                                skip_group_check=True,
                            )

                    encT_cur = encT_next

                    # ---- softmax over s (rows identical) ---------------
                    negmax = small.tile([128, 1], f32, tag="sm")
                    nc.vector.tensor_reduce(
                        negmax, att_ps, axis=AX, op=OPMAX, negate=True
                    )
                    exp_t = tanhp.tile([128, S], f32, tag="exp")
                    sumexp = small.tile([128, 1], f32, tag="sm")
                    nc.scalar.activation(
                        exp_t,
                        att_ps,
                        EXP,
                        bias=negmax,
                        scale=1.0,
                        accum_out=sumexp,
                    )
                    rsum = small.tile([128, 1], f32, tag="sm")
                    nc.vector.reciprocal(rsum, sumexp)
                    final_t = tanhp.tile([128, S], f32, tag="exp")
                    nc.vector.tensor_scalar_mul(final_t, exp_t, rsum)
                    nc.sync.dma_start(out=out[b : b + 1, :], in_=final_t[0:1, :])

    nc.compile()
    return nc


def _get_program(loop_iters=None, pe_dt=None):
    if pe_dt is None:
        pe_dt = PE_DT
    key = ("nc", loop_iters, pe_dt)
    if key not in _PROGRAM_CACHE:
        _PROGRAM_CACHE[key] = _build_program(loop_iters, pe_dt)
    return _PROGRAM_CACHE[key]


def _run(inputs, trace=False):
    from concourse.bass_utils import run_bass_kernel_spmd

    enc = np.ascontiguousarray(np.asarray(inputs["encoder_outputs"], np.float32))
    dec = np.ascontiguousarray(np.asarray(inputs["decoder_hidden"], np.float32))
    W = np.ascontiguousarray(np.asarray(inputs["W"], np.float32))
    b = np.ascontiguousarray(np.asarray(inputs["b"], np.float32))
    v = np.ascontiguousarray(np.asarray(inputs["v"], np.float32))

    nc = _get_program()
    in_maps = []
    for c in range(NCORES):
        sl = slice(c * BLOC, (c + 1) * BLOC)
        in_maps.append(
            {
                "enc": np.ascontiguousarray(enc[sl]),
                "dec": np.ascontiguousarray(dec[sl]),
                "W": W,
                "bias": b,
                "v": v,
            }
        )
    res = run_bass_kernel_spmd(
        nc, in_maps, core_ids=list(range(NCORES)), trace=trace
    )
    full = np.concatenate([r["out"] for r in res.results], axis=0)
    return full.astype(np.float32), res


def kernel(**inputs) -> np.ndarray:
    out, _ = _run(inputs, trace=False)
    return out


# revision 15
# speedup vs baseline: 1.7208x; 1.7208x over previous
"""Bahdanau-attention kernel for Trainium2 (8 NeuronCores, SPMD data-parallel).

Math (per batch b):
    dec_proj = decoder_hidden @ W[:1024] + b            # [A]
    z_T[a,s] = sum_e W[1024+e, a] * enc[b, s, e]        # [A, S] on PE
    energy   = tanh(z_T + dec_proj[:, None])            # ACT, per-partition bias
    logits   = sum_a v[a] * energy[a, s]                # PE matvec (v replicated)
    out[b]   = softmax(logits)                          # fp32 on ACT/DVE

Sharding: batch 32 -> 4 per core, weights replicated.
PE_DT picks the PE input dtype: "f16" (fp16, RNE ~2^-11) or "f32r" (FP22).
"""

import contextlib

import numpy as np

B, S, E, A, H = 32, 1024, 1024, 1024, 1024
NCORES = 8
BLOC = B // NCORES  # 4
KC = 8  # 128-chunks along each 1024-sized dim

PE_DT = "f16"

_PROGRAM_CACHE = {}


def _build_program(loop_iters=None, pe_dt=None):
    import concourse.bass as bass
    import concourse.tile as tile
    from concourse import bacc, mybir

    if pe_dt is None:
        pe_dt = PE_DT
    f32 = mybir.dt.float32
    f32r = mybir.dt.float32r
    f16 = mybir.dt.float16
    TANH = mybir.ActivationFunctionType.Tanh
    EXP = mybir.ActivationFunctionType.Exp
    AX = mybir.AxisListType.X
    OPMAX = mybir.AluOpType.max
    ts = bass.ts

    is16 = pe_dt == "f16"
    pdt = f16 if is16 else f32r

    nc = bacc.Bacc("TRN2", target_bir_lowering=False, debug=False)

    enc = nc.dram_tensor("enc", [BLOC, S, E], f32, kind="ExternalInput").ap()
    dec = nc.dram_tensor("dec", [BLOC, H], f32, kind="ExternalInput").ap()
    Wt = nc.dram_tensor("W", [H + E, A], f32, kind="ExternalInput").ap()
    bias = nc.dram_tensor("bias", [A], f32, kind="ExternalInput").ap()
    vvec = nc.dram_tensor("v", [A], f32, kind="ExternalInput").ap()
    out = nc.dram_tensor("out", [BLOC, S], f32, kind="ExternalOutput").ap()

    def w_src(ap2d):
        # fp16 path casts during SWDGE DMA; f32r path bitcasts bytes.
        return ap2d if is16 else ap2d.bitcast(f32r)

    def w_dma(o, i):
        if is16:
            nc.gpsimd.dma_start(out=o, in_=i)
        else:
            nc.sync.dma_start(out=o, in_=i)

    with tile.TileContext(nc) as tc:
        loop_cm = (
            tc.For_i(0, loop_iters, 1)
            if loop_iters is not None
            else contextlib.nullcontext()
        )
        with (
            tc.tile_pool(name="consts", bufs=1) as consts,
            tc.tile_pool(name="wenc", bufs=1) as wencp,
            tc.tile_pool(name="bigbuf", bufs=2) as bigbuf,
            tc.tile_pool(name="encT", bufs=2) as encTp,
            tc.tile_pool(name="tanh", bufs=3) as tanhp,
            tc.tile_pool(name="small", bufs=8) as small,
            tc.tile_pool(name="psz", bufs=3, space="PSUM") as pszp,
            tc.tile_pool(name="psatt", bufs=1, space="PSUM") as psattp,
        ):
            with loop_cm:
                # decT[p, h8, b] = dec[b, h8*128 + p]
                decT = consts.tile([128, KC, BLOC], pdt)
                dec_r = (dec if is16 else dec.bitcast(f32r)).rearrange(
                    "b (h8 p) -> p h8 b", p=128
                )
                for h8 in range(KC):
                    nc.gpsimd.dma_start(out=decT[:, h8, :], in_=dec_r[:, h8, :])
                # bias column per a-chunk: bcol[p, a8] = bias[a8*128 + p]
                bcol = consts.tile([128, KC], f32)
                nc.gpsimd.dma_start(
                    out=bcol, in_=bias.rearrange("(a8 p) -> p a8", p=128)
                )
                # v as columns, replicated across 128 cols per a-chunk
                vcol = consts.tile([128, KC], f32)
                nc.gpsimd.dma_start(
                    out=vcol, in_=vvec.rearrange("(a8 p) -> p a8", p=128)
                )
                ones = consts.tile([128, 128], f32)
                nc.vector.memset(ones, 1.0)
                # staging copy absorbs the DMA wait so tensor_scalar ops
                # below carry at most one sync-wait each
                vcol2 = consts.tile([128, KC], f32)
                nc.vector.tensor_copy(vcol2, vcol)
                vrep = consts.tile([128, KC, 128], pdt)
                for a8 in range(KC):
                    nc.vector.tensor_scalar_mul(
                        vrep[:, a8, :], ones, vcol2[:, a8 : a8 + 1]
                    )

                dproj = consts.tile([128, KC, BLOC], f32)

                # ---- batch-0 encoder load ------------------------------
                enc_src = enc if is16 else enc.bitcast(f32r)

                def enc_dma(o, i):
                    if is16:
                        nc.gpsimd.dma_start(out=o, in_=i)
                    else:
                        nc.sync.dma_start(out=o, in_=i)

                enc_nat = [None] * BLOC
                enc_nat[0] = bigbuf.tile(
                    [128, KC, E], pdt, tag="big", name="enc_nat0"
                )
                enc_r0 = enc_src[0].rearrange("(j p) e -> p j e", p=128)
                for s8 in range(KC):
                    enc_dma(enc_nat[0][:, s8, :], enc_r0[:, s8, :])

                # ---- W_enc ---------------------------------------------
                wenc = wencp.tile([128, KC, A], pdt)
                for e8 in range(KC):
                    w_dma(
                        wenc[:, e8, :],
                        w_src(Wt[H + e8 * 128 : H + (e8 + 1) * 128, :]),
                    )

                # ---- W_dec + dec_proj ----------------------------------
                if is16:
                    wdec = consts.tile([128, KC, A], pdt, name="wdec")
                else:
                    wdec = bigbuf.tile([128, KC, A], pdt, tag="big", name="wdec")
                for h8 in range(KC):
                    w_dma(wdec[:, h8, :], w_src(Wt[ts(h8, 128), :]))
                dp_ps = pszp.tile([128, KC, BLOC], f32, tag="z", name="dp_ps")
                for a8 in range(KC):
                    for h8 in range(KC):
                        nc.tensor.matmul(
                            dp_ps[:, a8, :],
                            wdec[:, h8, ts(a8, 128)],
                            decT[:, h8, :],
                            start=(h8 == 0),
                            stop=(h8 == KC - 1),
                        )
                for a8 in range(KC):
                    nc.vector.tensor_scalar_add(
                        dproj[:, a8, :], dp_ps[:, a8, :], bcol[:, a8 : a8 + 1]
                    )

                # ---- main batch loop -----------------------------------
                def emit_transpose_block(b, encT, s8):
                    # xbar DMA transpose: encT[p, j, s8*128+s'] =
                    #   enc_nat[b][s', s8, j*128+p]  (row e = j*128+p)
                    nc.sync.dma_start(
                        out=encT[:, :, ts(s8, 128)],
                        in_=enc_nat[b][:, s8, :],
                        transpose=True,
                    )

                def alloc_encT(b):
                    return encTp.tile(
                        [128, KC, S], pdt, tag="encT", name=f"eT{b}"
                    )

                # batch 0 transposes up front (chunk-JIT with its DMA)
                encT_cur = alloc_encT(0)
                for s8 in range(KC):
                    emit_transpose_block(0, encT_cur, s8)

                for b in range(BLOC):
                    if b + 1 < BLOC:
                        enc_nat[b + 1] = bigbuf.tile(
                            [128, KC, E], pdt, tag="big", name=f"enc_nat{b + 1}"
                        )
                        enc_rn = enc_src[b + 1].rearrange(
                            "(j p) e -> p j e", p=128
                        )
                        for s8 in range(KC):
                            enc_dma(enc_nat[b + 1][:, s8, :], enc_rn[:, s8, :])
                        encT_next = alloc_encT(b + 1)
                    else:
                        encT_next = None

                    att_ps = psattp.tile([128, S], f32, tag="att")

                    for a8 in range(KC):
                        # interleave next batch's transposes into this
                        # batch's z-phase (keeps PE dense / HAM warm)
                        if encT_next is not None:
                            emit_transpose_block(b + 1, encT_next, a8)
                        zp = pszp.tile([128, S], f32, tag="z")
                        for e8 in range(KC):
                            lhsT = wenc[:, e8, ts(a8, 128)]
                            nc.tensor.matmul(
                                zp[:, 0:512],
                                lhsT,
                                encT_cur[:, e8, 0:512],
                                start=(e8 == 0),
                                stop=(e8 == KC - 1),
                            )
                            nc.tensor.matmul(
                                zp[:, 512:1024],
                                lhsT,
                                encT_cur[:, e8, 512:1024],
                                start=(e8 == 0),
                                stop=(e8 == KC - 1),
                            )
                        tanh_t = tanhp.tile([128, S], pdt, tag="tanh")
                        nc.scalar.activation(
                            tanh_t,
                            zp,
                            TANH,
                            bias=dproj[:, a8, b : b + 1],
                            scale=1.0,
                        )
                        for half in (0, 512):
                            nc.tensor.matmul(
                                att_ps[:, half : half + 512],
                                vrep[:, a8, :],
                                tanh_t[:, half : half + 512],
                                start=(a8 == 0),
                                stop=(a8 == KC - 1),
                                skip_group_check=True,
                            )

                    encT_cur = encT_next

                    # ---- softmax over s (rows identical) ---------------
                    negmax = small.tile([128, 1], f32, tag="sm")
                    nc.vector.tensor_reduce(
                        negmax, att_ps, axis=AX, op=OPMAX, negate=True
                    )
                    exp_t = tanhp.tile([128, S], f32, tag="exp")
                    sumexp = small.tile([128, 1], f32, tag="sm")
                    nc.scalar.activation(
                        exp_t,
                        att_ps,
                        EXP,
                        bias=negmax,
                        scale=1.0,
                        accum_out=sumexp,
                    )
                    rsum = small.tile([128, 1], f32, tag="sm")
                    nc.vector.reciprocal(rsum, sumexp)
                    final_t = tanhp.tile([128, S], f32, tag="exp")
                    nc.vector.tensor_scalar_mul(final_t, exp_t, rsum)
                    nc.sync.dma_start(out=out[b : b + 1, :], in_=final_t[0:1, :])

    nc.compile()
    return nc


def _get_program(loop_iters=None, pe_dt=None):
    if pe_dt is None:
        pe_dt = PE_DT
    key = ("nc", loop_iters, pe_dt)
    if key not in _PROGRAM_CACHE:
        _PROGRAM_CACHE[key] = _build_program(loop_iters, pe_dt)
    return _PROGRAM_CACHE[key]


def _run(inputs, trace=False):
    from concourse.bass_utils import run_bass_kernel_spmd

    enc = np.ascontiguousarray(np.asarray(inputs["encoder_outputs"], np.float32))
    dec = np.ascontiguousarray(np.asarray(inputs["decoder_hidden"], np.float32))
    W = np.ascontiguousarray(np.asarray(inputs["W"], np.float32))
    b = np.ascontiguousarray(np.asarray(inputs["b"], np.float32))
    v = np.ascontiguousarray(np.asarray(inputs["v"], np.float32))

    nc = _get_program()
    in_maps = []
    for c in range(NCORES):
        sl = slice(c * BLOC, (c + 1) * BLOC)
        in_maps.append(
            {
                "enc": np.ascontiguousarray(enc[sl]),
                "dec": np.ascontiguousarray(dec[sl]),
                "W": W,
                "bias": b,
                "v": v,
            }
        )
    try:
        res = run_bass_kernel_spmd(
            nc, in_maps, core_ids=list(range(NCORES)), trace=trace
        )
    except Exception:
        # transient NRT device hiccups have been observed; retry once
        res = run_bass_kernel_spmd(
            nc, in_maps, core_ids=list(range(NCORES)), trace=trace
        )
    full = np.concatenate([r["out"] for r in res.results], axis=0)
    return full.astype(np.float32), res


def kernel(**inputs) -> np.ndarray:
    out, _ = _run(inputs, trace=False)
    return out
